# revision 44
# baseline (speedup 1.0000x reference)
"""CRsAE1d FISTA kernel for 8 Trainium2 NeuronCores.

Strategy
--------
H = [circ(f_0)|...|circ(f_7)] is block-circulant: with 128-row blocking each
circulant is block-bidiagonal with ONE repeated diagonal block D_k (lower-band
Toeplitz, f[0..63]) and ONE repeated subdiagonal block S_k (upper-corner band,
f[1..63]).  H@w / H^T@v are tiny [128,128] matmuls; the circular wrap is
handled by splitting each S-band matmul into a 120-column main part and an
8-column wrap part (no halo copies anywhere — matmul cost is proportional to
moving columns and the extra Ldweights dedups away).

Data-parallel over batch: 64 columns -> 8 cores x 8 columns.

Iteration state x lives ONLY in fp16 (bit-sim rel err 2.1e-3 vs the 2e-2
gate).  The momentum passthrough matmuls use (1+m_t)*I / -m_t*I weights so
PSUM directly accumulates the pre-shrink value c = w + (1/L)H^T v.  Shrink
x' = c - clamp(c, -thr, thr) runs per 2-filter chunk: chunks 0/2/3 as
clamp (tensor_scalar) + sub (tensor_tensor, fp16 out) on DVE, chunk 1 as
relu(c-thr) - relu(-c-thr) on the Scalar engine (which CAN read PSUM) with
the f16 combine on GpSimd (which cannot).  btmp for the next iteration is
computed on DVE in the shadow of conv2.  Final iteration writes f32 and
DMAs out per chunk.
"""

import sys

for p in ("/root/.axon_site", "/root/.axon_site/_ro/trn_rl_repo",
          "/root/.axon_site/_ro/pypackages", "/opt/trn_rl_repo"):
    if p not in sys.path:
        sys.path.append(p)

import numpy as np

T = 15
LAM = 0.1
N = 2048
K = 8
KS = 64
B = 64
NCORES = 8
BL = B // NCORES          # batch per core
NB = N // 128             # 16 row-blocks
CW = NB * BL              # 128 columns per (J,b) region
G = 2                     # filters per shrink chunk
NCH = K // G              # 4 chunks
GCW = G * CW              # 256

_CACHE: dict = {}


def _momentum_coeffs():
    s = 0.0
    ms = []
    for _ in range(T):
        st = (1.0 + np.sqrt(1.0 + 4.0 * s * s)) / 2.0
        ms.append(np.float32((s - 1.0) / st))
        s = st
    return ms


def _band_matrices(D):
    """D_k[r,s] = f_k[r-s] for 0<=r-s<KS;  S_k[r,s] = f_k[128+r-s] for s-r>=65."""
    Dm = np.zeros((K, 128, 128), np.float32)
    Sm = np.zeros((K, 128, 128), np.float32)
    r = np.arange(128)[:, None]
    s = np.arange(128)[None, :]
    d1 = r - s
    d2 = 128 + r - s
    m1 = (d1 >= 0) & (d1 < KS)
    m2 = (d2 > 0) & (d2 < KS)
    for k in range(K):
        Dm[k][m1] = D[k][d1[m1]]
        Sm[k][m2] = D[k][d2[m2]]
    return Dm, Sm


def _dedup_ldweights(d):
    """Remove Ldweights whose weight AP is identical to the previous PE
    weight load with only Matmults in between — the stationary operand is
    still in the array.  (bass emits one Ldweights per matmul, even for
    back-to-back matmuls sharing lhsT.)  Any waits on a removed Ldweights
    move onto the next PE instruction (the legalizer splits them later)."""
    for fn in d["functions"]:
        for bb in fn["blocks"]:
            out = []
            prev_key = None
            pending_waits = []
            for inst in bb["instructions"]:
                op = inst["opcode"]
                if op == "Ldweights":
                    w = inst["ins"][0]
                    key = (w.get("memref"), w.get("offset"), str(w.get("ap")),
                           str(inst.get("tile_position")))
                    si = inst.get("sync_info")
                    if key == prev_key:
                        if si and si.get("on_wait"):
                            pending_waits.extend(si["on_wait"])
                        assert not (si and si.get("on_update"))
                        continue
                    prev_key = key
                elif op == "Matmult":
                    if pending_waits:
                        si = inst.get("sync_info")
                        if si is None:
                            si = {"on_wait": [], "on_update": []}
                            inst["sync_info"] = si
                        si["on_wait"] = list(si.get("on_wait", [])) + pending_waits
                        pending_waits = []
                elif inst.get("engine") == "PE":
                    prev_key = None
                    if pending_waits:
                        si = inst.get("sync_info")
                        if si is None:
                            si = {"on_wait": [], "on_update": []}
                            inst["sync_info"] = si
                        si["on_wait"] = list(si.get("on_wait", [])) + pending_waits
                        pending_waits = []
                out.append(inst)
            assert not pending_waits
            bb["instructions"] = out
    return d


def _legalize_bir(bir_bytes):
    """The walrus build here encodes at most ONE sync-wait per instruction
    ("Too many sync wait commands").  Tile attaches up to 3.  Split the
    extras onto EventSemaphore wait-carrier instructions inserted just
    before, on the same engine (engine streams keep BB relative order, so
    the carriers execute immediately before the original)."""
    import orjson

    d = orjson.loads(bir_bytes)
    _dedup_ldweights(d)
    for fn in d["functions"]:
        for bb in fn["blocks"]:
            out = []
            for inst in bb["instructions"]:
                si = inst.get("sync_info")
                ow = si.get("on_wait", []) if si else []
                if len(ow) > 1:
                    for j, w in enumerate(ow[:-1]):
                        out.append({
                            "debug": inst.get("debug", 0),
                            "engine": inst["engine"],
                            "ins": [],
                            "outs": [],
                            "name": f"{inst['name']}_wsplit{j}",
                            "opcode": "EventSemaphore",
                            "sync_info": {"on_update": [], "on_wait": [w]},
                        })
                    si["on_wait"] = [ow[-1]]
                out.append(inst)
            bb["instructions"] = out
    return orjson.dumps(d)


def _install_patches():
    import concourse.bass2jax as b2j
    from concourse.bass_utils import compile_bir_kernel as _cbk

    def _cbk_legal(bir_str, compile_dir_path, neff_name):
        return _cbk(_legalize_bir(bir_str), compile_dir_path,
                    neff_name=neff_name)

    b2j.compile_bir_kernel = _cbk_legal


def _build_program():
    import concourse.bass as bass
    import concourse.mybir as mybir
    import concourse.tile as tile
    import bass_rust
    from concourse.tile import add_dep_helper as add_dep
    from concourse.vector_clock import ScopedClock

    _install_patches()

    # The nix walrus build rejects >1 sync-wait on CTRL-class (Drain)
    # instructions; split the Tile tail-drain waits across a chain of
    # single-wait drains.
    def _drain_and_barrier(self, tick_clock, wait_clock):
        drain_inst = self.nc.sync.drain()
        wait_clock.add_sem_waits(
            drain_inst.ins, ScopedClock({None: tick_clock.global_clock})
        )
        si = drain_inst.ins.sync_info
        waits = list(si.on_wait) if si is not None else []
        if len(waits) > 1:
            si.on_wait = waits[:1]
            for w in waits[1:]:
                d = self.nc.sync.drain()
                d.ins.sync_info = bass_rust.SyncInfo(on_wait=[w], on_update=[])
        self.nc.all_engine_barrier()
        assert self.sems is not None
        popped = self.nc._tile_sem_poison_stack.pop()
        assert popped is self._sem_poison
        self.nc.clear_and_free_semaphores(list(self.sems.allocated().values()))
        self.nc.all_engine_barrier()

    tile.TileContext._drain_and_barrier = _drain_and_barrier

    f32 = mybir.dt.float32
    f16 = mybir.dt.float16
    Alu = mybir.AluOpType
    Act = mybir.ActivationFunctionType
    ms = _momentum_coeffs()
    thr_f = float(_CACHE["thr"])

    nc = bass.Bass("TRN2", target_bir_lowering=False, debug=False,
                   num_devices=NCORES)
    d_sig = nc.dram_tensor("sig", [128, CW], f32, kind="ExternalInput").ap()
    # half-tensor weight regions: fully contiguous DRAM DMAs (fast) while
    # keeping t0/t1 pacing at 2-chunk granularity; more dispatches would
    # serialize on the ~610ns-per-dma_start SP queue cost
    d_w1h = [nc.dram_tensor(f"w1h{h}", [128, 8 * 128], f16,
                            kind="ExternalInput").ap() for h in range(2)]
    d_w2c = [nc.dram_tensor(f"w2c{c}", [128, 4 * 128], f16,
                            kind="ExternalInput").ap() for c in range(NCH)]
    d_eye = nc.dram_tensor("eye", [128, 128], f16, kind="ExternalInput").ap()
    d_outc = [nc.dram_tensor(f"xout{c}", [128, GCW], f32,
                             kind="ExternalOutput").ap() for c in range(NCH)]

    CONV1_ORDER = (0, 1, 2, 3)   # chunk readiness order at steady state

    with tile.TileContext(nc) as tc:
        with (
            tc.tile_pool(name="const", bufs=1) as const,
            tc.tile_pool(name="state", bufs=1) as state,
            tc.tile_pool(name="psq", bufs=2, space="PSUM") as psqp,
            tc.tile_pool(name="psu", bufs=1, space="PSUM") as psup,
            tc.tile_pool(name="vp", bufs=2) as vp,
            tc.tile_pool(name="clp", bufs=2) as clp,
            tc.tile_pool(name="junk", bufs=1, space="PSUM") as junkp,
        ):
            w1 = const.tile([128, 2 * K * 128], f16)
            w2 = const.tile([128, 2 * K * 128], f16)
            # one tile per iteration: per-tile dep tracking keeps the
            # Scalar-engine wid builds decoupled from PE momentum reads
            widt = [const.tile([128, 2 * 128], f16, name=f"wid{tp}")
                    for tp in range(T)]
            sigt = const.tile([128, CW], f32)
            warm = const.tile([128, 128], f16, name="warm")
            biast = const.tile([128, 1], f32, name="neg_thr")
            nc.gpsimd.memset(biast[:], -thr_f)
            HKW = K * 128  # half of a w tensor (k 0-3 / 4-7)
            # weight DMAs spread across queues; w2 chunk0 + sig first so
            # t=0 can start as early as possible
            # All weight transfers serialized on the SP queue in need-order
            # (sig gates t0's v16; w2 chunks pace t0's conv2; w1 chunks pace
            # t1's conv1) — parallel queues just steal DMA bandwidth from
            # the transfer the critical path is waiting on.  wid rides the
            # Scalar queue (2 dispatches, keeps the ACT table load early).
            eye16 = const.tile([128, 128], f16, name="eye16")
            nc.sync.dma_start(sigt[:], d_sig[:])
            nc.scalar.dma_start(eye16[:], d_eye[:])
            for c in range(NCH):
                nc.sync.dma_start(w2[:, 4 * c * 128:4 * (c + 1) * 128],
                                  d_w2c[c][:])
            for h in range(2):
                nc.sync.dma_start(w1[:, h * HKW:(h + 1) * HKW], d_w1h[h][:])

            def build_wid(tp):
                # wid_t = (1+m_t)*I | -m_t*I, built on the Scalar engine
                # from the fp16 identity (832KB of DMA saved)
                nc.scalar.activation(
                    widt[tp][:, 0:128], eye16[:],
                    Act.Copy, scale=float(1.0 + ms[tp]))
                if tp >= 2:
                    nc.scalar.activation(
                        widt[tp][:, 128:256],
                        eye16[:], Act.Copy, scale=float(-ms[tp]))

            # PE p-state pre-warm: ~2.5us of junk matmuls while the weight
            # DMAs are in flight, so t=0 runs at full clock
            nc.gpsimd.memset(warm[:], 0.0)
            junk = junkp.tile([128, CW], f32, tag="junk")
            for _ in range(60):
                nc.tensor.matmul(junk[:], warm[:], warm[:],
                                 start=True, stop=True)
            # wid for t=1,2 built up front (right after the eye DMA lands,
            # during the weight-DMA window)
            build_wid(1)
            build_wid(2)

            # x double-buffer, 4 chunk tiles each, [128, 2*CW] fp16 (no halo)
            X = [[state.tile([128, GCW], f16, name=f"X{a}c{c}")
                  for c in range(NCH)] for a in range(2)]
            btA = state.tile([128, CW], f32)
            btB = state.tile([128, CW], f32)
            xoutt = state.tile([128, K * CW], f32)

            for t in range(T):
                m = float(ms[t])
                Xc = X[t % 2]
                Xp = X[(t + 1) % 2]

                # conv1: psq = H @ x_t  (t>0; x_0 = 0)
                if t > 0:
                    psq = psqp.tile([128, CW], f32, tag="psq", name=f"psq{t}")
                    first = True
                    for ci, c in enumerate(CONV1_ORDER):
                        last_c = ci == NCH - 1
                        for g in range(G):
                            k = G * c + g
                            wD = w1[:, (2 * k) * 128:(2 * k + 1) * 128]
                            wS = w1[:, (2 * k + 1) * 128:(2 * k + 2) * 128]
                            xb = Xc[c][:, g * CW:(g + 1) * CW]
                            stop = last_c and g == G - 1
                            nc.tensor.matmul(psq[:], wD, xb,
                                             start=first, stop=False)
                            first = False
                            nc.tensor.matmul(psq[:, BL:CW], wS,
                                             xb[:, 0:CW - BL],
                                             start=False, stop=stop)
                            nc.tensor.matmul(psq[:, 0:BL], wS,
                                             xb[:, CW - BL:CW],
                                             start=False, stop=stop)

                # v = btmp - (1+m) q   (fp16)
                v16 = vp.tile([128, CW], f16, tag="v", name=f"v{t}")
                if t == 0:
                    nc.vector.tensor_copy(v16[:], sigt[:])
                else:
                    bt_cur = sigt if t <= 1 else (btA if t % 2 == 0 else btB)
                    nc.vector.scalar_tensor_tensor(
                        v16[:], psq[:], -(1.0 + m), bt_cur[:],
                        Alu.mult, Alu.add)
                    if t + 1 < T:
                        bt_next = btB if t % 2 == 0 else btA
                        nc.vector.scalar_tensor_tensor(
                            bt_next[:], psq[:], float(ms[t + 1]), sigt[:],
                            Alu.mult, Alu.add)

                # psu[c] = (1+m) x - m x_prev + (1/L) H^T v
                # (full 2KB PSUM bank per chunk: accumulation-group start
                #  flags are per-bank, chunks must not share banks)
                psuT = [psup.tile([128, 2 * GCW], f32, tag=f"psu{c}",
                                  name=f"psu{c}_{t}") for c in range(NCH)]
                psu = [p[:, 0:GCW] for p in psuT]

                def emit_mom(c):
                    if t == 0:
                        return
                    for g in range(G):
                        nc.tensor.matmul(
                            psu[c][:, g * CW:(g + 1) * CW],
                            widt[t][:, 0:128],
                            Xc[c][:, g * CW:(g + 1) * CW],
                            start=(g == 0), stop=False)
                    if t >= 2:
                        for g in range(G):
                            nc.tensor.matmul(
                                psu[c][:, g * CW:(g + 1) * CW],
                                widt[t][:, 128:256],
                                Xp[c][:, g * CW:(g + 1) * CW],
                                start=False, stop=False)

                def emit_conv2(c):
                    for g in range(G):
                        k = G * c + g
                        wD = w2[:, (2 * k) * 128:(2 * k + 1) * 128]
                        wS = w2[:, (2 * k + 1) * 128:(2 * k + 2) * 128]
                        reg = psu[c][:, g * CW:(g + 1) * CW]
                        stop = g == G - 1
                        nc.tensor.matmul(reg, wD, v16[:],
                                         start=(t == 0 and g == 0), stop=False)
                        nc.tensor.matmul(reg[:, 0:CW - BL], wS,
                                         v16[:, BL:CW],
                                         start=False, stop=stop)
                        nc.tensor.matmul(reg[:, CW - BL:CW], wS,
                                         v16[:, 0:BL],
                                         start=False, stop=stop)

                # interleave momentum + conv2 per chunk so psu[0] completes
                # as early as possible (cl0 is the head of the DVE chain)
                for c in range(NCH):
                    emit_mom(c)
                    emit_conv2(c)

                # shrink via softshrink(c) = min(c + thr, relu(c - thr)):
                # per chunk ONE Scalar relu (PSUM-capable) + ONE DVE
                # scalar_tensor_tensor (psu + thr) min a1 — the Scalar and
                # DVE stages pipeline across chunks.
                for c in range(NCH):
                    out_ap = (xoutt[:, c * GCW:(c + 1) * GCW] if t == T - 1
                              else Xp[c][:])
                    a1 = clp.tile([128, GCW], f32, tag=f"a1_{c}",
                                  name=f"a1_{c}_{t}")
                    nc.scalar.activation(a1[:], psu[c][:], Act.Relu,
                                         bias=biast[:, 0:1], scale=1.0)
                    nc.vector.scalar_tensor_tensor(
                        out_ap, psu[c][:], thr_f, a1[:],
                        Alu.add, Alu.min)
                    if t == T - 1:
                        nc.sync.dma_start(d_outc[c][:],
                                          xoutt[:, c * GCW:(c + 1) * GCW])

                # build momentum identity weights two iterations ahead on
                # the Scalar engine's idle time
                if 1 <= t < T - 2:
                    build_wid(t + 2)

    return nc


def kernel(signal, local_dictionary):
    sig = np.ascontiguousarray(np.asarray(signal, dtype=np.float32))
    D = np.ascontiguousarray(np.asarray(local_dictionary, dtype=np.float32))
    assert sig.shape == (N, B) and D.shape == (K, KS)

    # Lipschitz constant: H H^T = F^H diag(sum_k |fft(f_k)|^2) F  (circulants)
    fpad = np.zeros((K, N), np.float64)
    fpad[:, :KS] = D.astype(np.float64)
    L = np.float32((np.abs(np.fft.fft(fpad, axis=1)) ** 2).sum(0).max() + 1.0)
    thr = np.float32(LAM / L)
    _CACHE["thr"] = float(thr)

    Dm, Sm = _band_matrices(D)
    ms = _momentum_coeffs()

    # conv1 lhsT[j,i] = D_k[i,j]  (transposed);  conv2 lhsT[i,j] = D_k[i,j]/L
    w1 = np.empty((128, 2 * K * 128), np.float16)
    w2 = np.empty((128, 2 * K * 128), np.float16)
    for k in range(K):
        w1[:, (2 * k) * 128:(2 * k + 1) * 128] = Dm[k].T.astype(np.float16)
        w1[:, (2 * k + 1) * 128:(2 * k + 2) * 128] = Sm[k].T.astype(np.float16)
        w2[:, (2 * k) * 128:(2 * k + 1) * 128] = (Dm[k] / L).astype(np.float16)
        w2[:, (2 * k + 1) * 128:(2 * k + 2) * 128] = (Sm[k] / L).astype(np.float16)
    eye = np.eye(128, dtype=np.float32)

    nc = _build_program()

    from concourse.bass_utils import run_bass_kernel_spmd

    wmap = {}
    HKW = K * 128
    for h in range(2):
        wmap[f"w1h{h}"] = np.ascontiguousarray(w1[:, h * HKW:(h + 1) * HKW])
    for c in range(NCH):
        wmap[f"w2c{c}"] = np.ascontiguousarray(w2[:, 4 * c * 128:4 * (c + 1) * 128])
    wmap["eye"] = np.ascontiguousarray(eye.astype(np.float16))

    in_maps = []
    for c in range(NCORES):
        sc = sig[:, c * BL:(c + 1) * BL]                      # [2048, 8]
        sc = sc.reshape(NB, 128, BL).transpose(1, 0, 2).reshape(128, CW)
        in_maps.append({"sig": np.ascontiguousarray(sc), **wmap})

    _CACHE["in_maps"] = in_maps
    res = run_bass_kernel_spmd(nc, in_maps, list(range(NCORES)))

    out = np.empty((K * N, B), np.float32)
    for c in range(NCORES):
        xc = np.concatenate([res.results[c][f"xout{j}"] for j in range(NCH)],
                            axis=1)                           # [128, 1024]
        xc = xc.reshape(128, K, NB, BL).transpose(1, 2, 0, 3).reshape(K * N, BL)
        out[:, c * BL:(c + 1) * BL] = xc
    return out


# revision 46
# speedup vs baseline: 1.0184x; 1.0184x over previous
"""CRsAE1d FISTA kernel for 8 Trainium2 NeuronCores.

Strategy
--------
H = [circ(f_0)|...|circ(f_7)] is block-circulant: with 128-row blocking each
circulant is block-bidiagonal with ONE repeated diagonal block D_k (lower-band
Toeplitz, f[0..63]) and ONE repeated subdiagonal block S_k (upper-corner band,
f[1..63]).  H@w / H^T@v are tiny [128,128] matmuls; the circular wrap is
handled by splitting each S-band matmul into a 120-column main part and an
8-column wrap part (no halo copies anywhere — matmul cost is proportional to
moving columns and the extra Ldweights dedups away).

Data-parallel over batch: 64 columns -> 8 cores x 8 columns.

Iteration state x lives ONLY in fp16 (bit-sim rel err 2.1e-3 vs the 2e-2
gate).  The momentum passthrough matmuls use (1+m_t)*I / -m_t*I scaled
identities (built ON DEVICE from one fp16 eye via Scalar-engine scaled
copies, two iterations ahead — saves 832KB of startup DMA).  PSUM then
directly accumulates the pre-shrink value c = w + (1/L)H^T v.  Shrink uses
the identity softshrink(c) = min(c + thr, relu(c - thr)): per 2-filter
chunk ONE Scalar-engine relu (PSUM-capable) + ONE DVE scalar_tensor_tensor
((c + thr) min a1, fp16 out) — the two stages pipeline across chunks.
btmp for the next iteration is an stt on DVE in the shadow of conv2.
Final iteration writes f32 and DMAs out per chunk.
"""

import sys

for p in ("/root/.axon_site", "/root/.axon_site/_ro/trn_rl_repo",
          "/root/.axon_site/_ro/pypackages", "/opt/trn_rl_repo"):
    if p not in sys.path:
        sys.path.append(p)

import numpy as np

T = 15
LAM = 0.1
N = 2048
K = 8
KS = 64
B = 64
NCORES = 8
BL = B // NCORES          # batch per core
NB = N // 128             # 16 row-blocks
CW = NB * BL              # 128 columns per (J,b) region
G = 2                     # filters per shrink chunk
NCH = K // G              # 4 chunks
GCW = G * CW              # 256

_CACHE: dict = {}


def _momentum_coeffs():
    s = 0.0
    ms = []
    for _ in range(T):
        st = (1.0 + np.sqrt(1.0 + 4.0 * s * s)) / 2.0
        ms.append(np.float32((s - 1.0) / st))
        s = st
    return ms


def _band_matrices(D):
    """D_k[r,s] = f_k[r-s] for 0<=r-s<KS;  S_k[r,s] = f_k[128+r-s] for s-r>=65."""
    Dm = np.zeros((K, 128, 128), np.float32)
    Sm = np.zeros((K, 128, 128), np.float32)
    r = np.arange(128)[:, None]
    s = np.arange(128)[None, :]
    d1 = r - s
    d2 = 128 + r - s
    m1 = (d1 >= 0) & (d1 < KS)
    m2 = (d2 > 0) & (d2 < KS)
    for k in range(K):
        Dm[k][m1] = D[k][d1[m1]]
        Sm[k][m2] = D[k][d2[m2]]
    return Dm, Sm


def _dedup_ldweights(d):
    """Remove Ldweights whose weight AP is identical to the previous PE
    weight load with only Matmults in between — the stationary operand is
    still in the array.  (bass emits one Ldweights per matmul, even for
    back-to-back matmuls sharing lhsT.)  Any waits on a removed Ldweights
    move onto the next PE instruction (the legalizer splits them later)."""
    for fn in d["functions"]:
        for bb in fn["blocks"]:
            out = []
            prev_key = None
            pending_waits = []
            for inst in bb["instructions"]:
                op = inst["opcode"]
                if op == "Ldweights":
                    w = inst["ins"][0]
                    key = (w.get("memref"), w.get("offset"), str(w.get("ap")),
                           str(inst.get("tile_position")))
                    si = inst.get("sync_info")
                    if key == prev_key:
                        if si and si.get("on_wait"):
                            pending_waits.extend(si["on_wait"])
                        assert not (si and si.get("on_update"))
                        continue
                    prev_key = key
                elif op == "Matmult":
                    if pending_waits:
                        si = inst.get("sync_info")
                        if si is None:
                            si = {"on_wait": [], "on_update": []}
                            inst["sync_info"] = si
                        si["on_wait"] = list(si.get("on_wait", [])) + pending_waits
                        pending_waits = []
                elif inst.get("engine") == "PE":
                    prev_key = None
                    if pending_waits:
                        si = inst.get("sync_info")
                        if si is None:
                            si = {"on_wait": [], "on_update": []}
                            inst["sync_info"] = si
                        si["on_wait"] = list(si.get("on_wait", [])) + pending_waits
                        pending_waits = []
                out.append(inst)
            assert not pending_waits
            bb["instructions"] = out
    return d


def _legalize_bir(bir_bytes):
    """The walrus build here encodes at most ONE sync-wait per instruction
    ("Too many sync wait commands").  Tile attaches up to 3.  Split the
    extras onto EventSemaphore wait-carrier instructions inserted just
    before, on the same engine (engine streams keep BB relative order, so
    the carriers execute immediately before the original)."""
    import orjson

    d = orjson.loads(bir_bytes)
    _dedup_ldweights(d)
    for fn in d["functions"]:
        for bb in fn["blocks"]:
            out = []
            for inst in bb["instructions"]:
                si = inst.get("sync_info")
                ow = si.get("on_wait", []) if si else []
                if len(ow) > 1:
                    for j, w in enumerate(ow[:-1]):
                        out.append({
                            "debug": inst.get("debug", 0),
                            "engine": inst["engine"],
                            "ins": [],
                            "outs": [],
                            "name": f"{inst['name']}_wsplit{j}",
                            "opcode": "EventSemaphore",
                            "sync_info": {"on_update": [], "on_wait": [w]},
                        })
                    si["on_wait"] = [ow[-1]]
                out.append(inst)
            bb["instructions"] = out
    return orjson.dumps(d)


def _install_patches():
    import concourse.bass2jax as b2j
    from concourse.bass_utils import compile_bir_kernel as _cbk

    def _cbk_legal(bir_str, compile_dir_path, neff_name):
        return _cbk(_legalize_bir(bir_str), compile_dir_path,
                    neff_name=neff_name)

    b2j.compile_bir_kernel = _cbk_legal


def _build_program():
    import concourse.bass as bass
    import concourse.mybir as mybir
    import concourse.tile as tile
    import bass_rust
    from concourse.tile import add_dep_helper as add_dep
    from concourse.vector_clock import ScopedClock

    _install_patches()

    # The nix walrus build rejects >1 sync-wait on CTRL-class (Drain)
    # instructions; split the Tile tail-drain waits across a chain of
    # single-wait drains.
    def _drain_and_barrier(self, tick_clock, wait_clock):
        drain_inst = self.nc.sync.drain()
        wait_clock.add_sem_waits(
            drain_inst.ins, ScopedClock({None: tick_clock.global_clock})
        )
        si = drain_inst.ins.sync_info
        waits = list(si.on_wait) if si is not None else []
        if len(waits) > 1:
            si.on_wait = waits[:1]
            for w in waits[1:]:
                d = self.nc.sync.drain()
                d.ins.sync_info = bass_rust.SyncInfo(on_wait=[w], on_update=[])
        self.nc.all_engine_barrier()
        assert self.sems is not None
        popped = self.nc._tile_sem_poison_stack.pop()
        assert popped is self._sem_poison
        self.nc.clear_and_free_semaphores(list(self.sems.allocated().values()))
        self.nc.all_engine_barrier()

    tile.TileContext._drain_and_barrier = _drain_and_barrier

    f32 = mybir.dt.float32
    f16 = mybir.dt.float16
    Alu = mybir.AluOpType
    Act = mybir.ActivationFunctionType
    ms = _momentum_coeffs()
    thr_f = float(_CACHE["thr"])

    nc = bass.Bass("TRN2", target_bir_lowering=False, debug=False,
                   num_devices=NCORES)
    d_sig = nc.dram_tensor("sig", [128, CW], f32, kind="ExternalInput").ap()
    # half-tensor weight regions: fully contiguous DRAM DMAs (fast) while
    # keeping t0/t1 pacing at 2-chunk granularity; more dispatches would
    # serialize on the ~610ns-per-dma_start SP queue cost
    d_w1h = [nc.dram_tensor(f"w1h{h}", [128, 8 * 128], f16,
                            kind="ExternalInput").ap() for h in range(2)]
    d_w2h = [nc.dram_tensor(f"w2h{h}", [128, 8 * 128], f16,
                            kind="ExternalInput").ap() for h in range(2)]
    d_eye = nc.dram_tensor("eye", [128, 128], f16, kind="ExternalInput").ap()
    d_outc = [nc.dram_tensor(f"xout{c}", [128, GCW], f32,
                             kind="ExternalOutput").ap() for c in range(NCH)]

    CONV1_ORDER = (0, 1, 2, 3)   # chunk readiness order at steady state

    with tile.TileContext(nc) as tc:
        with (
            tc.tile_pool(name="const", bufs=1) as const,
            tc.tile_pool(name="state", bufs=1) as state,
            tc.tile_pool(name="psq", bufs=2, space="PSUM") as psqp,
            tc.tile_pool(name="psu", bufs=1, space="PSUM") as psup,
            tc.tile_pool(name="vp", bufs=2) as vp,
            tc.tile_pool(name="clp", bufs=2) as clp,
            tc.tile_pool(name="junk", bufs=1, space="PSUM") as junkp,
        ):
            w1 = const.tile([128, 2 * K * 128], f16)
            w2 = const.tile([128, 2 * K * 128], f16)
            # one tile per iteration: per-tile dep tracking keeps the
            # Scalar-engine wid builds decoupled from PE momentum reads
            widt = [const.tile([128, 2 * 128], f16, name=f"wid{tp}")
                    for tp in range(T)]
            sigt = const.tile([128, CW], f32)
            warm = const.tile([128, 128], f16, name="warm")
            biast = const.tile([128, 1], f32, name="neg_thr")
            nc.gpsimd.memset(biast[:], -thr_f)
            HKW = K * 128  # half of a w tensor (k 0-3 / 4-7)
            # weight DMAs spread across queues; w2 chunk0 + sig first so
            # t=0 can start as early as possible
            # All weight transfers serialized on the SP queue in need-order
            # (sig gates t0's v16; w2 chunks pace t0's conv2; w1 chunks pace
            # t1's conv1) — parallel queues just steal DMA bandwidth from
            # the transfer the critical path is waiting on.  wid rides the
            # Scalar queue (2 dispatches, keeps the ACT table load early).
            eye16 = const.tile([128, 128], f16, name="eye16")
            nc.sync.dma_start(sigt[:], d_sig[:])
            nc.scalar.dma_start(eye16[:], d_eye[:])
            for h in range(2):
                nc.sync.dma_start(w2[:, h * HKW:(h + 1) * HKW], d_w2h[h][:])
            for h in range(2):
                nc.sync.dma_start(w1[:, h * HKW:(h + 1) * HKW], d_w1h[h][:])

            def build_wid(tp):
                # wid_t = (1+m_t)*I | -m_t*I, built on the Scalar engine
                # from the fp16 identity (832KB of DMA saved)
                nc.scalar.activation(
                    widt[tp][:, 0:128], eye16[:],
                    Act.Copy, scale=float(1.0 + ms[tp]))
                if tp >= 2:
                    nc.scalar.activation(
                        widt[tp][:, 128:256],
                        eye16[:], Act.Copy, scale=float(-ms[tp]))

            # PE p-state pre-warm: ~2.5us of junk matmuls while the weight
            # DMAs are in flight, so t=0 runs at full clock
            nc.gpsimd.memset(warm[:], 0.0)
            junk = junkp.tile([128, CW], f32, tag="junk")
            for _ in range(60):
                nc.tensor.matmul(junk[:], warm[:], warm[:],
                                 start=True, stop=True)
            # wid for t=1,2 built up front (right after the eye DMA lands,
            # during the weight-DMA window)
            build_wid(1)
            build_wid(2)

            # x double-buffer, 4 chunk tiles each, [128, 2*CW] fp16 (no halo)
            X = [[state.tile([128, GCW], f16, name=f"X{a}c{c}")
                  for c in range(NCH)] for a in range(2)]
            btA = state.tile([128, CW], f32)
            btB = state.tile([128, CW], f32)
            xoutt = state.tile([128, K * CW], f32)

            for t in range(T):
                m = float(ms[t])
                Xc = X[t % 2]
                Xp = X[(t + 1) % 2]

                # conv1: psq = H @ x_t  (t>0; x_0 = 0)
                if t > 0:
                    psq = psqp.tile([128, CW], f32, tag="psq", name=f"psq{t}")
                    first = True
                    for ci, c in enumerate(CONV1_ORDER):
                        last_c = ci == NCH - 1
                        for g in range(G):
                            k = G * c + g
                            wD = w1[:, (2 * k) * 128:(2 * k + 1) * 128]
                            wS = w1[:, (2 * k + 1) * 128:(2 * k + 2) * 128]
                            xb = Xc[c][:, g * CW:(g + 1) * CW]
                            stop = last_c and g == G - 1
                            nc.tensor.matmul(psq[:], wD, xb,
                                             start=first, stop=False)
                            first = False
                            nc.tensor.matmul(psq[:, BL:CW], wS,
                                             xb[:, 0:CW - BL],
                                             start=False, stop=stop)
                            nc.tensor.matmul(psq[:, 0:BL], wS,
                                             xb[:, CW - BL:CW],
                                             start=False, stop=stop)

                # v = btmp - (1+m) q   (fp16)
                v16 = vp.tile([128, CW], f16, tag="v", name=f"v{t}")
                if t == 0:
                    nc.vector.tensor_copy(v16[:], sigt[:])
                else:
                    bt_cur = sigt if t <= 1 else (btA if t % 2 == 0 else btB)
                    nc.vector.scalar_tensor_tensor(
                        v16[:], psq[:], -(1.0 + m), bt_cur[:],
                        Alu.mult, Alu.add)
                    if t + 1 < T:
                        bt_next = btB if t % 2 == 0 else btA
                        nc.vector.scalar_tensor_tensor(
                            bt_next[:], psq[:], float(ms[t + 1]), sigt[:],
                            Alu.mult, Alu.add)

                # psu[c] = (1+m) x - m x_prev + (1/L) H^T v
                # (full 2KB PSUM bank per chunk: accumulation-group start
                #  flags are per-bank, chunks must not share banks)
                psuT = [psup.tile([128, 2 * GCW], f32, tag=f"psu{c}",
                                  name=f"psu{c}_{t}") for c in range(NCH)]
                psu = [p[:, 0:GCW] for p in psuT]

                def emit_mom(c):
                    if t == 0:
                        return
                    for g in range(G):
                        nc.tensor.matmul(
                            psu[c][:, g * CW:(g + 1) * CW],
                            widt[t][:, 0:128],
                            Xc[c][:, g * CW:(g + 1) * CW],
                            start=(g == 0), stop=False)
                    if t >= 2:
                        for g in range(G):
                            nc.tensor.matmul(
                                psu[c][:, g * CW:(g + 1) * CW],
                                widt[t][:, 128:256],
                                Xp[c][:, g * CW:(g + 1) * CW],
                                start=False, stop=False)

                def emit_conv2(c):
                    for g in range(G):
                        k = G * c + g
                        wD = w2[:, (2 * k) * 128:(2 * k + 1) * 128]
                        wS = w2[:, (2 * k + 1) * 128:(2 * k + 2) * 128]
                        reg = psu[c][:, g * CW:(g + 1) * CW]
                        stop = g == G - 1
                        nc.tensor.matmul(reg, wD, v16[:],
                                         start=(t == 0 and g == 0), stop=False)
                        nc.tensor.matmul(reg[:, 0:CW - BL], wS,
                                         v16[:, BL:CW],
                                         start=False, stop=stop)
                        nc.tensor.matmul(reg[:, CW - BL:CW], wS,
                                         v16[:, 0:BL],
                                         start=False, stop=stop)

                # interleave momentum + conv2 per chunk so psu[0] completes
                # as early as possible (cl0 is the head of the DVE chain)
                for c in range(NCH):
                    emit_mom(c)
                    emit_conv2(c)

                # shrink via softshrink(c) = min(c + thr, relu(c - thr)):
                # per chunk ONE Scalar relu (PSUM-capable) + ONE DVE
                # scalar_tensor_tensor (psu + thr) min a1 — the Scalar and
                # DVE stages pipeline across chunks.
                for c in range(NCH):
                    out_ap = (xoutt[:, c * GCW:(c + 1) * GCW] if t == T - 1
                              else Xp[c][:])
                    a1 = clp.tile([128, GCW], f32, tag=f"a1_{c}",
                                  name=f"a1_{c}_{t}")
                    nc.scalar.activation(a1[:], psu[c][:], Act.Relu,
                                         bias=biast[:, 0:1], scale=1.0)
                    nc.vector.scalar_tensor_tensor(
                        out_ap, psu[c][:], thr_f, a1[:],
                        Alu.add, Alu.min)
                    if t == T - 1:
                        eng = nc.scalar if c in (1, 2) else nc.sync
                        eng.dma_start(d_outc[c][:],
                                      xoutt[:, c * GCW:(c + 1) * GCW])

                # build momentum identity weights two iterations ahead on
                # the Scalar engine's idle time
                if 1 <= t < T - 2:
                    build_wid(t + 2)

    return nc


def kernel(signal, local_dictionary):
    sig = np.ascontiguousarray(np.asarray(signal, dtype=np.float32))
    D = np.ascontiguousarray(np.asarray(local_dictionary, dtype=np.float32))
    assert sig.shape == (N, B) and D.shape == (K, KS)

    # Lipschitz constant: H H^T = F^H diag(sum_k |fft(f_k)|^2) F  (circulants)
    fpad = np.zeros((K, N), np.float64)
    fpad[:, :KS] = D.astype(np.float64)
    L = np.float32((np.abs(np.fft.fft(fpad, axis=1)) ** 2).sum(0).max() + 1.0)
    thr = np.float32(LAM / L)
    _CACHE["thr"] = float(thr)

    Dm, Sm = _band_matrices(D)
    ms = _momentum_coeffs()

    # conv1 lhsT[j,i] = D_k[i,j]  (transposed);  conv2 lhsT[i,j] = D_k[i,j]/L
    w1 = np.empty((128, 2 * K * 128), np.float16)
    w2 = np.empty((128, 2 * K * 128), np.float16)
    for k in range(K):
        w1[:, (2 * k) * 128:(2 * k + 1) * 128] = Dm[k].T.astype(np.float16)
        w1[:, (2 * k + 1) * 128:(2 * k + 2) * 128] = Sm[k].T.astype(np.float16)
        w2[:, (2 * k) * 128:(2 * k + 1) * 128] = (Dm[k] / L).astype(np.float16)
        w2[:, (2 * k + 1) * 128:(2 * k + 2) * 128] = (Sm[k] / L).astype(np.float16)
    eye = np.eye(128, dtype=np.float32)

    nc = _build_program()

    from concourse.bass_utils import run_bass_kernel_spmd

    wmap = {}
    HKW = K * 128
    for h in range(2):
        wmap[f"w1h{h}"] = np.ascontiguousarray(w1[:, h * HKW:(h + 1) * HKW])
        wmap[f"w2h{h}"] = np.ascontiguousarray(w2[:, h * HKW:(h + 1) * HKW])
    wmap["eye"] = np.ascontiguousarray(eye.astype(np.float16))

    in_maps = []
    for c in range(NCORES):
        sc = sig[:, c * BL:(c + 1) * BL]                      # [2048, 8]
        sc = sc.reshape(NB, 128, BL).transpose(1, 0, 2).reshape(128, CW)
        in_maps.append({"sig": np.ascontiguousarray(sc), **wmap})

    _CACHE["in_maps"] = in_maps
    res = run_bass_kernel_spmd(nc, in_maps, list(range(NCORES)))

    out = np.empty((K * N, B), np.float32)
    for c in range(NCORES):
        xc = np.concatenate([res.results[c][f"xout{j}"] for j in range(NCH)],
                            axis=1)                           # [128, 1024]
        xc = xc.reshape(128, K, NB, BL).transpose(1, 2, 0, 3).reshape(K * N, BL)
        out[:, c * BL:(c + 1) * BL] = xc
    return out
